# revision 1
# baseline (speedup 1.0000x reference)
"""Causal multi-head attention on 8 Trainium2 NeuronCores.

Problem: x[4, 2048, 1024], 16 heads of 64; q/k/v = x@W* + b*, causal
softmax attention, out = y@Wp + bp.

Sharding: core c handles batch b = c//2 and head-group hg = c%2
(8 heads = 512 feature columns of Wq/Wk/Wv, 512 rows of Wp).  Each core
computes a full [2048, 1024] partial of the output projection for its
batch; the host sums the two partials per batch and adds bp.

Per-core dataflow (all matmuls float32r, PSUM accumulation fp32), built
so attention overlaps the projections:
  * x is transposed (PE) and projected in FOUR 512-row sequence
    quarters through a double-buffered quarter-sized xT; causality
    means q-block `seg`'s attention only needs k/v prefixes already
    produced, so each quarter's attention streams while the next
    quarter projects.
  * qT/kT [512, t] via interleaved m-chunks of Wk/Wq; v [t, 512] stored
    per-head with an appended ones column (softmax denominators fall
    out of the same AV matmul, row 64).
  * per (head, q-block 512, k-block 128): sT = exp(0.125 * kT_h.T@qT_h)
    straight from PSUM on ACT (no max-subtraction; scores are O(1));
    triangular 0/1 mask on the single diagonal 128x128 block;
    y_extT[65, q] += v_ext.T @ sT accumulated in PSUM.
  * y is written IN PLACE over qT (head h's own rows/columns are dead
    once its scores are done); softmax normalization uses a DVE
    reciprocal from PSUM + DRAM-roundtrip partition broadcast, per
    head-pair, overlapping later heads.
  * out[t 128, d 512] = yT.T @ Wp -> DRAM.
"""
import numpy as np

B, T, D = 4, 2048, 1024
NH, HD = 16, 64
NHL = 8            # heads per core
DL = NHL * HD      # 512: local qkv feature width
P = 128
QB = 512           # q block (columns of sT tiles)
NQ = T // QB       # 4
NKT = T // P       # 16 k blocks
KC = D // P        # 8 contraction chunks over model dim
FC = DL // P       # 4 chunks over local feature dim
DB = 512           # out-projection column block
ND = D // DB       # 2
TH = T // 2        # 1024: sequence half

_CACHE = {}


def _build():
    import concourse.bass as bass
    from concourse import bacc
    import concourse.mybir as mybir
    import concourse.tile as tile

    f32 = mybir.dt.float32
    f32r = mybir.dt.float32r
    Exp = mybir.ActivationFunctionType.Exp
    Copy = mybir.ActivationFunctionType.Copy

    nc = bacc.Bacc(None)
    x_d = nc.dram_tensor("x", [T, D], f32r, kind="ExternalInput")
    wq_d = nc.dram_tensor("wq", [D, DL], f32r, kind="ExternalInput")
    wk_d = nc.dram_tensor("wk", [D, DL], f32r, kind="ExternalInput")
    wv_d = nc.dram_tensor("wv", [D, DL], f32r, kind="ExternalInput")
    wp_d = nc.dram_tensor("wp", [DL, D], f32r, kind="ExternalInput")
    bq_d = nc.dram_tensor("bq", [DL], f32, kind="ExternalInput")
    bk_d = nc.dram_tensor("bk", [DL], f32, kind="ExternalInput")
    bv_d = nc.dram_tensor("bv", [DL], f32r, kind="ExternalInput")
    mask_d = nc.dram_tensor("mask", [P, P], f32r, kind="ExternalInput")
    ident_d = nc.dram_tensor("ident", [P, P], f32r, kind="ExternalInput")
    out_d = nc.dram_tensor("out", [T, D], f32r, kind="ExternalOutput")

    def bcast_ap(ap, parts):
        """Partition-broadcast view of a DRAM AP (stride-0 partition dim)."""
        return bass.AP(tensor=ap.tensor, offset=ap.offset,
                       ap=[[0, parts]] + list(ap.ap))

    with tile.TileContext(nc) as tc:
        with (
            tc.tile_pool(name="const", bufs=1) as const,
            tc.tile_pool(name="qkT", bufs=1) as qkT_pool,
            tc.tile_pool(name="vext", bufs=1) as v_pool,
            tc.tile_pool(name="sums", bufs=1) as sums_pool,
            tc.tile_pool(name="dram", bufs=1, space="DRAM") as dram_pool,
        ):
            ident = const.tile([P, P], f32r)
            nc.sync.dma_start(ident, ident_d.ap())
            mask_sb = const.tile([P, P], f32r)
            nc.sync.dma_start(mask_sb, mask_d.ap())
            bq_sb = const.tile([P, FC], f32)
            nc.sync.dma_start(bq_sb, bq_d.ap().rearrange("(c p) -> p c", p=P))
            bk_sb = const.tile([P, FC], f32)
            nc.sync.dma_start(bk_sb, bk_d.ap().rearrange("(c p) -> p c", p=P))
            bv_sb = const.tile([P, DL], f32r)
            nc.gpsimd.dma_start(out=bv_sb, in_=bcast_ap(bv_d.ap(), P))

            qT_sb = qkT_pool.tile([P, FC, T], f32r)   # becomes yT in place
            kT_sb = qkT_pool.tile([P, FC, T], f32r)
            v_sb = v_pool.tile([P, NKT, NHL, HD + 1], f32r)
            ones_sb = const.tile([P, NKT, NHL], f32)
            nc.vector.memset(ones_sb, 1.0)
            nc.vector.tensor_copy(v_sb[:, :, :, HD], ones_sb)
            # head h softmax 1/sums at partition (h%4)*32; head pairs
            # alternate halves of the buffer (compute APs must start at a
            # multiple of 32 partitions).
            sums_sb = sums_pool.tile([P, 2, T], f32)
            rec_dram = dram_pool.tile([NHL, T], f32)

            with (
                tc.tile_pool(name="xT", bufs=2) as xT_pool,
                tc.tile_pool(name="xin", bufs=2) as xin,
                tc.tile_pool(name="wv", bufs=1) as wv_pool,
                tc.tile_pool(name="wqk", bufs=2) as wqk_pool,
                tc.tile_pool(name="sT", bufs=2) as sT_pool,
                tc.tile_pool(name="rbc", bufs=1) as rbc_pool,
                tc.tile_pool(name="ps_t", bufs=2, space="PSUM") as ps_t,
                tc.tile_pool(name="ps_m", bufs=1, space="PSUM") as ps_m,
                tc.tile_pool(name="ps_s", bufs=3, space="PSUM") as ps_s,
                tc.tile_pool(name="ps_y", bufs=2, space="PSUM") as ps_y,
            ):

                def attention(h, jq):
                    hp = (h % 2) * HD
                    hc = h // 2
                    q0 = jq * QB
                    psy = ps_y.tile([P, QB], f32)
                    n_ik = 4 * jq + 4
                    for ik in range(n_ik):
                        pd = ik - 4 * jq
                        c0 = max(0, pd * P)
                        pss = ps_s.tile([P, QB], f32)
                        nc.tensor.matmul(
                            pss[:, c0:QB],
                            lhsT=kT_sb[hp:hp + HD, hc, ik * P:(ik + 1) * P],
                            rhs=qT_sb[hp:hp + HD, hc, q0 + c0:q0 + QB],
                            start=True, stop=True)
                        sT = sT_pool.tile([P, QB], f32r)
                        nc.scalar.activation(
                            out=sT[:, c0:QB], in_=pss[:, c0:QB],
                            func=Exp, scale=0.125)
                        if pd >= 0:
                            nc.vector.tensor_mul(
                                sT[:, c0:c0 + P], sT[:, c0:c0 + P], mask_sb)
                        nc.tensor.matmul(
                            psy[0:HD + 1, c0:QB],
                            lhsT=v_sb[:, ik, h, :],
                            rhs=sT[:, c0:QB],
                            start=(ik == 0), stop=(ik == n_ik - 1))
                    # y written in place over this head's dead qT columns
                    nc.vector.tensor_copy(
                        qT_sb[hp:hp + HD, hc, q0:q0 + QB], psy[0:HD, :])
                    sp = (h % 4) * 32
                    nc.vector.reciprocal(
                        sums_sb[sp:sp + 1, h // 4, q0:q0 + QB],
                        psy[HD:HD + 1, :])

                wv_sb = wv_pool.tile([P, KC, DL], f32r)

                def transpose_seg(jt0, xT_sb):
                    for jt in range(jt0, jt0 + NQ):
                        xt = xin.tile([P, D], f32r)
                        nc.sync.dma_start(xt,
                                          x_d.ap()[jt * P:(jt + 1) * P, :])
                        for c in range(KC):
                            pt = ps_t.tile([P, P], f32r)
                            nc.tensor.transpose(
                                pt, xt[:, c * P:(c + 1) * P], ident)
                            dst_ap = xT_sb[:, c,
                                           (jt - jt0) * P:(jt - jt0 + 1) * P]
                            if (jt * KC + c) % 4 != 0:
                                nc.vector.tensor_copy(dst_ap, pt)
                            else:
                                nc.scalar.activation(out=dst_ap, in_=pt,
                                                     func=Copy)

                def v_seg(jt0, xT_sb):
                    for jt in range(jt0, jt0 + NQ):
                        ps = ps_m.tile([P, DL], f32, tag="m")
                        for kc in range(KC):
                            nc.tensor.matmul(
                                ps,
                                lhsT=xT_sb[:, kc,
                                           (jt - jt0) * P:(jt - jt0 + 1) * P],
                                rhs=wv_sb[:, kc, :],
                                start=(kc == 0), stop=(kc == KC - 1))
                        nc.vector.tensor_tensor(
                            v_sb[:, jt, :, 0:HD],
                            ps.rearrange("p (h e) -> p h e", h=NHL),
                            bv_sb.rearrange("p (h e) -> p h e", h=NHL),
                            mybir.AluOpType.add)

                def qk_chunk(seg, m, w_d, b_sb, dst, xT_sb):
                    w_sb = wqk_pool.tile([P, KC, P], f32r, tag="wqk")
                    nc.sync.dma_start(
                        w_sb,
                        w_d.ap().rearrange("(c p) m -> p c m", p=P)
                        [:, :, m * P:(m + 1) * P])
                    ps = ps_m.tile([P, QB], f32, tag="m")
                    for kc in range(KC):
                        nc.tensor.matmul(
                            ps,
                            lhsT=w_sb[:, kc, :],
                            rhs=xT_sb[:, kc, :],
                            start=(kc == 0), stop=(kc == KC - 1))
                    nc.vector.tensor_scalar_add(
                        dst[:, m, seg * QB:(seg + 1) * QB], ps,
                        b_sb[:, m:m + 1])

                def normalize_pair(h):
                    c = h // 2
                    for hh in (h - 1, h):
                        sph = (hh % 4) * 32
                        nc.sync.dma_start(
                            rec_dram[hh:hh + 1, :],
                            sums_sb[sph:sph + 1, hh // 4, :])
                    r_sb = rbc_pool.tile([P, T], f32)
                    nc.gpsimd.dma_start(out=r_sb[0:HD, :],
                                        in_=bcast_ap(rec_dram[h - 1], HD))
                    nc.gpsimd.dma_start(out=r_sb[HD:P, :],
                                        in_=bcast_ap(rec_dram[h], HD))
                    nc.vector.tensor_mul(
                        qT_sb[:, c, :], qT_sb[:, c, :], r_sb)

                # Quarter-wise streaming: prep x/v/qk for q-block `seg`,
                # then that block's attention for all heads — which overlaps
                # the next quarter's prep (causal: block seg only needs
                # k/v prefixes already produced).
                for seg in range(NQ):
                    jt0 = seg * NQ
                    xT_sb = xT_pool.tile([P, KC, QB], f32r)
                    transpose_seg(jt0, xT_sb)
                    if seg == 0:
                        # load Wv after the first x tiles are queued so the
                        # cold-start transposes aren't stuck behind 2MB
                        nc.sync.dma_start(
                            wv_sb,
                            wv_d.ap().rearrange("(c p) m -> p c m", p=P))
                    v_seg(jt0, xT_sb)
                    for m in range(FC):
                        for w_d, b_sb, dst in ((wk_d, bk_sb, kT_sb),
                                               (wq_d, bq_sb, qT_sb)):
                            qk_chunk(seg, m, w_d, b_sb, dst, xT_sb)
                    for h in range(NHL):
                        attention(h, seg)
                        if seg == NQ - 1 and h % 2 == 1:
                            normalize_pair(h)

            # ---- output projection (yT lives in qT_sb) ----
            with (
                tc.tile_pool(name="wp", bufs=1) as wp_pool,
                tc.tile_pool(name="outsb", bufs=8) as out_pool,
                tc.tile_pool(name="psD", bufs=8, space="PSUM") as psD,
            ):
                wp_sb = wp_pool.tile([P, FC, D], f32r)
                nc.sync.dma_start(
                    wp_sb, wp_d.ap().rearrange("(c p) m -> p c m", p=P))
                for jt in range(NKT):
                    for nd in range(ND):
                        ps = psD.tile([P, DB], f32)
                        for c in range(FC):
                            nc.tensor.matmul(
                                ps,
                                lhsT=qT_sb[:, c, jt * P:(jt + 1) * P],
                                rhs=wp_sb[:, c, nd * DB:(nd + 1) * DB],
                                start=(c == 0), stop=(c == FC - 1))
                        ot = out_pool.tile([P, DB], f32r)
                        if (jt * ND + nd) % 2 == 0:
                            nc.vector.tensor_copy(ot, ps)
                        else:
                            nc.scalar.activation(out=ot, in_=ps, func=Copy)
                        nc.sync.dma_start(
                            out_d.ap()[jt * P:(jt + 1) * P,
                                       nd * DB:(nd + 1) * DB],
                            ot)

    nc.finalize()
    return nc


def _in_maps(x, Wq, bq, Wk, bk, Wv, bv, Wp):
    mask = np.triu(np.ones((P, P), dtype=np.float32))  # keep q >= k
    maps = []
    for c in range(8):
        b, hg = divmod(c, 2)
        sl = slice(hg * DL, (hg + 1) * DL)
        maps.append({
            "x": np.ascontiguousarray(x[b]),
            "wq": np.ascontiguousarray(Wq[:, sl]),
            "wk": np.ascontiguousarray(Wk[:, sl]),
            "wv": np.ascontiguousarray(Wv[:, sl]),
            "wp": np.ascontiguousarray(Wp[sl, :]),
            "bq": np.ascontiguousarray(bq[sl]),
            "bk": np.ascontiguousarray(bk[sl]),
            "bv": np.ascontiguousarray(bv[sl]),
            "mask": mask,
            "ident": np.eye(P, dtype=np.float32),
        })
    return maps


def kernel(x, Wq, bq, Wk, bk, Wv, bv, Wp, bp):
    from concourse.bass_utils import run_bass_kernel_spmd

    if "nc" not in _CACHE:
        _CACHE["nc"] = _build()
    nc = _CACHE["nc"]

    x = np.asarray(x, np.float32)
    Wq, bq, Wk, bk, Wv, bv, Wp = [
        np.asarray(a, np.float32) for a in (Wq, bq, Wk, bk, Wv, bv, Wp)]
    bp = np.asarray(bp, np.float32)

    in_maps = _in_maps(x, Wq, bq, Wk, bk, Wv, bv, Wp)
    _CACHE["in_maps"] = in_maps

    res = run_bass_kernel_spmd(nc, in_maps, list(range(8))).results
    out = np.empty((B, T, D), dtype=np.float32)
    for b in range(B):
        out[b] = res[2 * b]["out"] + res[2 * b + 1]["out"] + bp
    return out

